# revision 22
# baseline (speedup 1.0000x reference)
"""MoE layer (B=8192, D=2048, H=2048, E=8, top-2) on 8 TRN2 NeuronCores.

Strategy: expert-parallel with host-side routing, tokens on the matmul FREE
dim.  The reference baseline put tokens on the PSUM partition dim, which
forces 128-token granularity per expert segment: sum_e ceil(c_e/128) = 133
blocks -> 17 blocks (2176 rows) on the critical core, a 232us PE floor.
Putting tokens on the free dim makes PE cost additive in tokens, so expert
segments can have arbitrary sizes.

One SPMD program must serve all 8 cores, so every core gets the same K
segment sizes (s_1..s_K, sum=T); a small DP picks sizes and an assignment
matrix A[e,j] (# cores whose slot j holds expert e, column sums = 8) with
sum_j A[e,j]*s_j >= c_e.  For the seed-0 input this gives T=2054 (6 pad
tokens/core), K=3: PE floor 219us vs the old 232us.

Per core: y[h, t] = sum_d W_e(t)[h, d] * x[t, d] computed as 16 h-chunks x
16 k-chunks x ~6 token windows; x (8.4MB bf16) is SBUF-resident, W streamed
per h-chunk from a host-prepacked [16, 128, 2048] layout (4KB DMA rows),
y written back transposed in bf16.  Phase A scans h over slot 0's windows
only (1 W stream) while the rest of x lands; phase B scans h over the
remaining slots' windows.
"""

import math

import numpy as np

B, D, H, E, TOPK = 8192, 2048, 2048, 8, 2
NCORES = 8
P = 128
KO = D // P  # 16 contraction sub-tiles
HC = H // P  # 16 h-chunks
KG = 4       # ko per DMA group
NG = KO // KG

# test.py flips TRACE to profile HW exec time; grading leaves it False.
TRACE = False
WARMUP = 22  # PE warm-up dummies (HAM clock ramp) bridging the DMA-fill window
# First FP8_KO of the 16 contraction sub-tiles run in fp8-e4m3 DoubleRow
# (2 sub-tiles per PE instruction = 2x bf16 FLOP rate, measured on HW).
# Even values only. 4 -> 12.5% less PE time; end-to-end rel err 0.0165
# (vs 0.0026 pure-bf16) against the 2e-2 gate.
FP8_KO = 4
last_exec_time_ns = None
last_trace_path = None

# Memoized plan for the seed-0 reference input (counts -> (T, sizes, assign)).
# assign[e][j] = number of cores whose slot j is assigned expert e.
_KNOWN_PLANS = {
    (2123, 1956, 2095, 2063, 2060, 1900, 1992, 2195): (
        2054,
        (634, 688, 732),
        (
            (0, 1, 2),
            (2, 1, 0),
            (1, 0, 2),
            (0, 3, 0),
            (0, 3, 0),
            (3, 0, 0),
            (2, 0, 1),
            (0, 0, 3),
        ),
    ),
}


def _routing(x, gate_W, gate_b):
    """Reference-exact gating on jax-CPU: logits -> top_k -> softmax."""
    import jax
    import jax.numpy as jnp

    with jax.default_device(jax.devices("cpu")[0]):
        logits = jnp.asarray(x) @ jnp.asarray(gate_W).T + jnp.asarray(gate_b)
        topk_vals, topk_idx = jax.lax.top_k(logits, TOPK)
        topk_w = jax.nn.softmax(topk_vals, axis=1)
    return np.asarray(topk_idx), np.asarray(topk_w, dtype=np.float32)


def _dp_cover(sizes, caps, maxpad_total):
    """Find per-expert piece counts n[e][j] with sum_j n*s_j in
    [caps[e], caps[e]+pad], column sums exactly 8, total pad exactly
    8*T - sum(caps).  Returns assign tuple or None."""
    K = len(sizes)
    opts = []
    for c in caps:
        o = []

        def rec(j, used, vec):
            if j == K - 1:
                lo = max(0, -(-(c - used) // sizes[j]))
                hi = min(8, (c + maxpad_total - used) // sizes[j])
                for n in range(lo, hi + 1):
                    pad = used + n * sizes[j] - c
                    if pad >= 0:
                        o.append((tuple(vec + [n]), pad))
                return
            for n in range(0, 9):
                u = used + n * sizes[j]
                if u > c + maxpad_total:
                    break
                rec(j + 1, u, vec + [n])

        rec(0, 0, [])
        if not o:
            return None
        opts.append(o)
    states = {(0,) * K + (0,): []}
    for e in range(len(caps)):
        new = {}
        for st, hist in states.items():
            cols, tot = st[:K], st[K]
            for v, pad in opts[e]:
                if tot + pad > maxpad_total:
                    continue
                ns = tuple(a + b for a, b in zip(cols, v))
                if any(x > 8 for x in ns):
                    continue
                key = ns + (tot + pad,)
                if key not in new:
                    new[key] = hist + [v]
        states = new
        if not states:
            return None
    for st, hist in states.items():
        if st[:K] == (8,) * K and st[K] == maxpad_total:
            return tuple(tuple(v) for v in hist)
    return None


def _plan_slots(counts):
    """Pick (T, sizes, assign): uniform per-core slot sizes summing to T and
    an expert->pieces assignment covering every expert's token count."""
    key = tuple(int(c) for c in counts)
    if key in _KNOWN_PLANS:
        return _KNOWN_PLANS[key]
    total = int(sum(counts))
    t_lo = -(-total // 8)
    for T in range(t_lo, t_lo + 65):
        maxpad = 8 * T - total
        if maxpad < 0:
            continue
        # K=2
        for s1 in range(256, T // 2 + 1):
            r = _dp_cover((s1, T - s1), counts, maxpad)
            if r:
                return T, (s1, T - s1), r
        # K=3
        for s1 in range(256, T // 3 + 1):
            for s2 in range(s1, (T - s1) // 2 + 1):
                s3 = T - s1 - s2
                if s3 < s2:
                    continue
                r = _dp_cover((s1, s2, s3), counts, maxpad)
                if r:
                    return T, (s1, s2, s3), r
    # Fallback: one expert per core, padded to the max count (needs E == 8).
    T = int(max(counts))
    return T, (T,), tuple((1,) for _ in counts)


def _windows(sizes):
    """Per-slot token windows (slot, t0, width), each <= 512 wide and
    single-expert by construction."""
    wins = []
    off = 0
    for j, s in enumerate(sizes):
        nw = -(-s // 512)
        base, rem = divmod(s, nw)
        o = off
        for i in range(nw):
            w = base + (1 if i < rem else 0)
            wins.append((j, o, w))
            o += w
        off += s
    return wins


def _build_bass(T, sizes, descale):
    import concourse.bacc as bacc
    import concourse.mybir as mybir
    import concourse.tile as tile

    bf16, f32 = mybir.dt.bfloat16, mybir.dt.float32
    f8 = mybir.dt.float8e4
    K = len(sizes)
    wins = _windows(sizes)
    wins_by_slot = [[w for w in wins if w[0] == j] for j in range(K)]
    XG = 1
    Q8 = FP8_KO // 2          # fp8 DoubleRow pairs per window
    G0 = FP8_KO // KG         # first bf16 W ko-group
    XG0 = FP8_KO // XG        # first bf16 x ko-group

    nc = bacc.Bacc("TRN2", target_bir_lowering=False)
    xT = nc.dram_tensor("xT", [D, T], bf16, kind="ExternalInput")
    ws = [
        nc.dram_tensor(f"w{j}", [HC, P, H], bf16, kind="ExternalInput")
        for j in range(K)
    ]
    if FP8_KO:
        x8T = nc.dram_tensor("x8T", [FP8_KO * P, T], f8, kind="ExternalInput")
        w8s = [
            nc.dram_tensor(f"w8_{j}", [HC, P, FP8_KO * P], f8, kind="ExternalInput")
            for j in range(K)
        ]
    y = nc.dram_tensor("y", [HC, P, T], bf16, kind="ExternalOutput")

    with tile.TileContext(nc) as tc:
        with (
            tc.tile_pool(name="warm", bufs=1) as warm,
            tc.tile_pool(name="xpool", bufs=1) as xpool,
            tc.tile_pool(name="wpool", bufs=3) as wpool,
            tc.tile_pool(name="ypool", bufs=4) as ypool,
            tc.tile_pool(name="psum", bufs=1, space="PSUM") as psum,
        ):
            # PE warm-up: dummies with no DMA deps run during the initial
            # fill window so HAM ramps the clock before real matmuls.
            wa = warm.tile([P, P], bf16)
            nc.vector.memset(wa[:], 0.0)
            nps = 4 if FP8_KO else 6
            pss = [
                psum.tile([P, 512], f32, name=f"ps{i}", tag=f"ps{i}")
                for i in range(nps)
            ]
            ps8s = [
                psum.tile([P, 512], f32, name=f"ps8_{i}", tag=f"ps8_{i}")
                for i in range(4 if FP8_KO else 0)
            ]
            for i in range(WARMUP):
                nc.tensor.matmul(
                    pss[i % nps][:, :P], wa[:], wa[:], start=True, stop=True
                )

            xr = xT.rearrange("(ko p) t -> p ko t", p=P)
            if FP8_KO:
                x8r = x8T.rearrange("(ko p) t -> p ko t", p=P)
            xt, x8t = {}, {}

            def x_issues_for_slot(j):
                """DMA-issue thunks for slot j's resident x tiles (bf16
                ko-groups >= XG0, plus fp8 pair tiles)."""
                issues = []
                for _, t0, w in wins_by_slot[j]:
                    for q in range(Q8):
                        def issue(t0=t0, w=w, q=q):
                            tl = xpool.tile([P, 2, w], f8, tag=f"x8_{t0}_{q}")
                            nc.sync.dma_start(
                                tl[:], x8r[:, 2 * q : 2 * q + 2, t0 : t0 + w]
                            )
                            x8t[(t0, q)] = tl
                        issues.append(issue)
                    for g in range(XG0, KO // XG):
                        def issue(t0=t0, w=w, g=g):
                            tl = xpool.tile([P, XG, w], bf16, tag=f"x_{t0}_{g}")
                            nc.sync.dma_start(
                                tl[:], xr[:, g * XG : (g + 1) * XG, t0 : t0 + w]
                            )
                            xt[(t0, g)] = tl
                        issues.append(issue)
                return issues

            rot = [0]
            y_pending = []

            def do_piece(h, t0, off, pw, wt, wt8, y_tag):
                """Matmuls + eviction + writeback for token range
                [t0+off, t0+off+pw) of the window starting at t0."""
                r = rot[0] % len(pss)
                rot[0] += 1
                ps = pss[r]
                if FP8_KO:
                    ps8 = ps8s[r]
                    for q in range(Q8):
                        nc.tensor.matmul(
                            ps8[:, :pw],
                            wt8[:, 2 * q : 2 * q + 2, :],
                            x8t[(t0, q)][:, :, off : off + pw],
                            start=(q == 0),
                            stop=(q == Q8 - 1),
                            perf_mode=mybir.MatmulPerfMode.DoubleRow,
                        )
                for ko in range(FP8_KO, KO):
                    nc.tensor.matmul(
                        ps[:, :pw],
                        wt[ko // KG][:, (ko % KG) * P : (ko % KG + 1) * P],
                        xt[(t0, ko // XG)][:, ko % XG, off : off + pw],
                        start=(ko == FP8_KO),
                        stop=(ko == KO - 1),
                    )
                if y_tag is None:
                    yt = ypool.tile([P, 512], bf16, tag="y")
                else:
                    yt = ypool.tile([P, pw], bf16, tag=y_tag)
                if FP8_KO:
                    nc.vector.tensor_scalar_mul(yt[:, :pw], ps8[:, :pw], descale)
                    nc.vector.tensor_tensor(
                        yt[:, :pw], yt[:, :pw], ps[:, :pw], mybir.AluOpType.add
                    )
                else:
                    nc.vector.tensor_copy(yt[:, :pw], ps[:, :pw])
                if y_tag is None:
                    nc.scalar.dma_start(y[h, :, t0 + off : t0 + off + pw], yt[:, :pw])
                else:
                    # Phase 0 is DMA-saturated (~270GB/s): park its y in
                    # SBUF and flush during phase 1, which has headroom.
                    y_pending.append((h, t0 + off, pw, yt))

            def do_windows(h, j, wt, wt8, buffer_y=False, split_last=False):
                wl = wins_by_slot[j]
                for wi, (_, t0, w) in enumerate(wl):
                    y_tag = f"y0_{h}_{wi}" if buffer_y else None
                    if split_last and wi == len(wl) - 1:
                        # Halve the final window so the first half's eviction
                        # and writeback overlap the second half's matmuls.
                        nc_half = w // 2
                        do_piece(h, t0, 0, nc_half, wt, wt8, y_tag)
                        do_piece(h, t0, nc_half, w - nc_half, wt, wt8, y_tag)
                    else:
                        do_piece(h, t0, 0, w, wt, wt8, y_tag)

            def flush_y(n):
                for _ in range(n):
                    if y_pending:
                        fh, ft0, fw, fyt = y_pending.pop(0)
                        nc.scalar.dma_start(y[fh, :, ft0 : ft0 + fw], fyt[:, :fw])

            def load_w(j, h):
                tls = {}
                for g in range(G0, NG):
                    tl = wpool.tile([P, KG * P], bf16, tag=f"w{j}_{g}")
                    nc.gpsimd.dma_start(
                        tl[:], ws[j][h, :, g * KG * P : (g + 1) * KG * P]
                    )
                    tls[g] = tl
                return tls

            def load_w8(j, h):
                tl = wpool.tile([P, FP8_KO, P], f8, tag=f"w8_{j}")
                nc.gpsimd.dma_start(
                    tl[:],
                    w8s[j][h].rearrange("p (ko hi) -> p ko hi", hi=P),
                )
                return tl

            # Slot 0's x tiles are needed immediately; later slots' issues are
            # spread across the back half of the preceding slot's h-scan so
            # the early queues carry only what the current phase consumes.
            for issue in x_issues_for_slot(0):
                issue()
            DEFER_H = 6  # first h-step of a phase that issues next-slot x
            for j in range(K):
                pending = x_issues_for_slot(j + 1) if j + 1 < K else []
                per_h = -(-len(pending) // (HC - DEFER_H)) if pending else 0
                n_flush = -(-len(y_pending) // HC) if j == 1 else 0
                for h in range(HC):
                    wt8 = load_w8(j, h) if FP8_KO else None
                    wt = load_w(j, h)
                    do_windows(
                        h, j, wt, wt8,
                        buffer_y=(j == 0 and K > 1),
                        split_last=(j == K - 1 and h == HC - 1),
                    )
                    flush_y(n_flush)
                    if h >= DEFER_H:
                        for _ in range(per_h):
                            if pending:
                                pending.pop(0)()
            flush_y(len(y_pending))
    nc.compile()
    return nc


def _install_profshim():
    """Register the NTFF profile hook trn_boot couldn't (image's antenv lacks
    axon_hooks) and stub the S3 artifact upload. Only needed when TRACE."""
    import sys
    import types

    import antenv

    if "antenv.axon_hooks" not in sys.modules:
        mod = types.ModuleType("antenv.axon_hooks")
        _hook = [None]
        mod.set_axon_ntff_profile_hook = lambda h: _hook.__setitem__(0, h)
        mod.get_axon_ntff_profile_hook = lambda: _hook[0]
        sys.modules["antenv.axon_hooks"] = mod
        antenv.axon_hooks = mod
        from trn_agent_boot.trn_boot import _ntff_profile_via_ctypes

        mod.set_axon_ntff_profile_hook(
            _ntff_profile_via_ctypes("/opt/axon/libaxon_pjrt.so")
        )
    import concourse.bass_utils as _bu

    _bu.upload_artifacts = lambda tmpdir: f"local:{tmpdir}"


def kernel(x, expert_W, expert_b, gate_W, gate_b):
    global last_exec_time_ns, last_trace_path
    import ml_dtypes

    from concourse.bass_utils import run_bass_kernel_spmd

    x = np.asarray(x, dtype=np.float32)
    expert_W = np.asarray(expert_W, dtype=np.float32)
    expert_b = np.asarray(expert_b, dtype=np.float32)
    gate_W = np.asarray(gate_W, dtype=np.float32)
    gate_b = np.asarray(gate_b, dtype=np.float32)

    topk_idx, topk_w = _routing(x, gate_W, gate_b)

    # Dispatch: token lists per expert (each token appears in exactly TOPK).
    tok = [np.nonzero((topk_idx == e).any(axis=1))[0] for e in range(E)]
    counts = np.array([len(t) for t in tok])
    T, sizes, assign = _plan_slots(counts)
    # Largest slot first: its h-scan phase is the longest window over which
    # the startup DMA transient (x + W streams) can hide.
    order = sorted(range(len(sizes)), key=lambda j: -sizes[j])
    sizes = tuple(sizes[j] for j in order)
    assign = tuple(tuple(row[j] for j in order) for row in assign)
    K = len(sizes)
    offs = np.concatenate([[0], np.cumsum(sizes)])

    # Slot -> expert per core: slot j's 8 core-slots are dealt to experts
    # per assign[:, j].
    slot_expert = np.zeros((NCORES, K), dtype=np.int64)
    for j in range(K):
        lst = [e for e in range(E) for _ in range(assign[e][j])]
        assert len(lst) == NCORES
        for c in range(NCORES):
            slot_expert[c, j] = lst[c]

    bf16 = ml_dtypes.bfloat16
    f8 = ml_dtypes.float8_e4m3
    xb = x.astype(bf16)
    # Power-of-2 fp8 scales sized so absmax lands just under e4m3 max (240).
    DC = FP8_KO * P
    if FP8_KO:
        sx = 2.0 ** math.floor(math.log2(224.0 / float(np.abs(x[:, :DC]).max())))
        sw = 2.0 ** math.floor(
            math.log2(224.0 / float(np.abs(expert_W[:, :, :DC]).max()))
        )
        descale = 1.0 / (sx * sw)
    else:
        sx = sw = descale = 1.0
    # W prepack per expert: [ho, p, ko*128+hi] = W_e[ho*128+hi, ko*128+p]
    wpack = [
        np.ascontiguousarray(
            expert_W[e]
            .reshape(HC, P, KO, P)
            .transpose(0, 3, 2, 1)
            .reshape(HC, P, H)
            .astype(bf16)
        )
        for e in range(E)
    ]
    w8pack = [
        np.ascontiguousarray(
            (expert_W[e][:, :DC] * sw)
            .reshape(HC, P, FP8_KO, P)
            .transpose(0, 3, 2, 1)
            .reshape(HC, P, DC)
            .astype(f8)
        )
        for e in range(E)
    ] if FP8_KO else None

    xTs = [np.zeros((D, T), dtype=bf16) for _ in range(NCORES)]
    x8s = [np.zeros((DC, T), dtype=f8) for _ in range(NCORES)] if FP8_KO else None
    core_of = np.zeros((E, B), dtype=np.int64)
    pos_of = np.zeros((E, B), dtype=np.int64)
    for e in range(E):
        pieces = [
            (c, j) for j in range(K) for c in range(NCORES) if slot_expert[c, j] == e
        ]
        cum = 0
        for c, j in pieces:
            n = min(int(sizes[j]), len(tok[e]) - cum)
            if n <= 0:
                continue
            t = tok[e][cum : cum + n]
            lo = int(offs[j])
            xTs[c][:, lo : lo + n] = xb[t].T
            if FP8_KO:
                x8s[c][:, lo : lo + n] = (x[t, :DC] * sx).astype(f8).T
            core_of[e, t] = c
            pos_of[e, t] = lo + np.arange(n)
            cum += n
        assert cum == len(tok[e]), f"expert {e}: assigned {cum} of {len(tok[e])}"

    in_maps = []
    for c in range(NCORES):
        m = {"xT": xTs[c]}
        for j in range(K):
            m[f"w{j}"] = wpack[slot_expert[c, j]]
            if FP8_KO:
                m[f"w8_{j}"] = w8pack[slot_expert[c, j]]
        if FP8_KO:
            m["x8T"] = x8s[c]
        in_maps.append(m)

    if TRACE:
        _install_profshim()
    nc = _build_bass(T, sizes, descale)
    res = run_bass_kernel_spmd(nc, in_maps, list(range(NCORES)), trace=TRACE)
    last_exec_time_ns = res.exec_time_ns
    if res.instructions_and_trace:
        last_trace_path = res.instructions_and_trace[1]

    # y [HC, P, T] -> [T, H] fp32 per core
    Ys = np.stack(
        [
            np.ascontiguousarray(
                np.asarray(res.results[c]["y"]).transpose(2, 0, 1)
            ).reshape(T, H).astype(np.float32)
            for c in range(NCORES)
        ]
    )

    barange = np.arange(B)
    out = np.zeros((B, H), dtype=np.float32)
    for k in range(TOPK):
        ek = topk_idx[:, k]
        out += topk_w[:, k, None] * (
            Ys[core_of[ek, barange], pos_of[ek, barange], :] + expert_b[ek]
        )
    return out


# revision 23
# speedup vs baseline: 1.0180x; 1.0180x over previous
"""MoE layer (B=8192, D=2048, H=2048, E=8, top-2) on 8 TRN2 NeuronCores.

Strategy: expert-parallel with host-side routing, tokens on the matmul FREE
dim.  The reference baseline put tokens on the PSUM partition dim, which
forces 128-token granularity per expert segment: sum_e ceil(c_e/128) = 133
blocks -> 17 blocks (2176 rows) on the critical core, a 232us PE floor.
Putting tokens on the free dim makes PE cost additive in tokens, so expert
segments can have arbitrary sizes.

One SPMD program must serve all 8 cores, so every core gets the same K
segment sizes (s_1..s_K, sum=T); a small DP picks sizes and an assignment
matrix A[e,j] (# cores whose slot j holds expert e, column sums = 8) with
sum_j A[e,j]*s_j >= c_e.  For the seed-0 input this gives T=2054 (6 pad
tokens/core), K=3: PE floor 219us vs the old 232us.

Per core: y[h, t] = sum_d W_e(t)[h, d] * x[t, d] computed as 16 h-chunks x
16 k-chunks x ~6 token windows; x (8.4MB bf16) is SBUF-resident, W streamed
per h-chunk from a host-prepacked [16, 128, 2048] layout (4KB DMA rows),
y written back transposed in bf16.  One h-scan phase per slot (largest
first) so each phase carries a single W stream; later slots' x tiles and
phase-0's buffered y writebacks are deferred into phases with DMA headroom.
The first FP8_KO contraction sub-tiles run as fp8-e4m3 DoubleRow matmuls
(2x bf16 FLOP rate) into a separate PSUM, descaled and combined at
eviction.
"""

import math

import numpy as np

B, D, H, E, TOPK = 8192, 2048, 2048, 8, 2
NCORES = 8
P = 128
KO = D // P  # 16 contraction sub-tiles
HC = H // P  # 16 h-chunks
KG = 4       # ko per DMA group
NG = KO // KG

# test.py flips TRACE to profile HW exec time; grading leaves it False.
TRACE = False
WARMUP = 22  # PE warm-up dummies (HAM clock ramp) bridging the DMA-fill window
# First FP8_KO of the 16 contraction sub-tiles run in fp8-e4m3 DoubleRow
# (2 sub-tiles per PE instruction = 2x bf16 FLOP rate, measured on HW).
# Even values only. 4 -> 12.5% less PE time; end-to-end rel err 0.0165
# (vs 0.0026 pure-bf16) against the 2e-2 gate.
FP8_KO = 4
last_exec_time_ns = None
last_trace_path = None

# Memoized plan for the seed-0 reference input (counts -> (T, sizes, assign)).
# assign[e][j] = number of cores whose slot j is assigned expert e.
_KNOWN_PLANS = {
    (2123, 1956, 2095, 2063, 2060, 1900, 1992, 2195): (
        2054,
        (634, 688, 732),
        (
            (0, 1, 2),
            (2, 1, 0),
            (1, 0, 2),
            (0, 3, 0),
            (0, 3, 0),
            (3, 0, 0),
            (2, 0, 1),
            (0, 0, 3),
        ),
    ),
}


def _routing(x, gate_W, gate_b):
    """Reference-exact gating on jax-CPU: logits -> top_k -> softmax."""
    import jax
    import jax.numpy as jnp

    with jax.default_device(jax.devices("cpu")[0]):
        logits = jnp.asarray(x) @ jnp.asarray(gate_W).T + jnp.asarray(gate_b)
        topk_vals, topk_idx = jax.lax.top_k(logits, TOPK)
        topk_w = jax.nn.softmax(topk_vals, axis=1)
    return np.asarray(topk_idx), np.asarray(topk_w, dtype=np.float32)


def _dp_cover(sizes, caps, maxpad_total):
    """Find per-expert piece counts n[e][j] with sum_j n*s_j in
    [caps[e], caps[e]+pad], column sums exactly 8, total pad exactly
    8*T - sum(caps).  Returns assign tuple or None."""
    K = len(sizes)
    opts = []
    for c in caps:
        o = []

        def rec(j, used, vec):
            if j == K - 1:
                lo = max(0, -(-(c - used) // sizes[j]))
                hi = min(8, (c + maxpad_total - used) // sizes[j])
                for n in range(lo, hi + 1):
                    pad = used + n * sizes[j] - c
                    if pad >= 0:
                        o.append((tuple(vec + [n]), pad))
                return
            for n in range(0, 9):
                u = used + n * sizes[j]
                if u > c + maxpad_total:
                    break
                rec(j + 1, u, vec + [n])

        rec(0, 0, [])
        if not o:
            return None
        opts.append(o)
    states = {(0,) * K + (0,): []}
    for e in range(len(caps)):
        new = {}
        for st, hist in states.items():
            cols, tot = st[:K], st[K]
            for v, pad in opts[e]:
                if tot + pad > maxpad_total:
                    continue
                ns = tuple(a + b for a, b in zip(cols, v))
                if any(x > 8 for x in ns):
                    continue
                key = ns + (tot + pad,)
                if key not in new:
                    new[key] = hist + [v]
        states = new
        if not states:
            return None
    for st, hist in states.items():
        if st[:K] == (8,) * K and st[K] == maxpad_total:
            return tuple(tuple(v) for v in hist)
    return None


def _plan_slots(counts):
    """Pick (T, sizes, assign): uniform per-core slot sizes summing to T and
    an expert->pieces assignment covering every expert's token count."""
    key = tuple(int(c) for c in counts)
    if key in _KNOWN_PLANS:
        return _KNOWN_PLANS[key]
    total = int(sum(counts))
    t_lo = -(-total // 8)
    for T in range(t_lo, t_lo + 65):
        maxpad = 8 * T - total
        if maxpad < 0:
            continue
        # K=2
        for s1 in range(256, T // 2 + 1):
            r = _dp_cover((s1, T - s1), counts, maxpad)
            if r:
                return T, (s1, T - s1), r
        # K=3
        for s1 in range(256, T // 3 + 1):
            for s2 in range(s1, (T - s1) // 2 + 1):
                s3 = T - s1 - s2
                if s3 < s2:
                    continue
                r = _dp_cover((s1, s2, s3), counts, maxpad)
                if r:
                    return T, (s1, s2, s3), r
    # Fallback: one expert per core, padded to the max count (needs E == 8).
    T = int(max(counts))
    return T, (T,), tuple((1,) for _ in counts)


def _windows(sizes):
    """Per-slot token windows (slot, t0, width), each <= 512 wide and
    single-expert by construction."""
    wins = []
    off = 0
    for j, s in enumerate(sizes):
        nw = -(-s // 512)
        base, rem = divmod(s, nw)
        o = off
        for i in range(nw):
            w = base + (1 if i < rem else 0)
            wins.append((j, o, w))
            o += w
        off += s
    return wins


def _build_bass(T, sizes, descale):
    import concourse.bacc as bacc
    import concourse.mybir as mybir
    import concourse.tile as tile

    bf16, f32 = mybir.dt.bfloat16, mybir.dt.float32
    f8 = mybir.dt.float8e4
    K = len(sizes)
    wins = _windows(sizes)
    wins_by_slot = [[w for w in wins if w[0] == j] for j in range(K)]
    XG = 1
    Q8 = FP8_KO // 2          # fp8 DoubleRow pairs per window
    G0 = FP8_KO // KG         # first bf16 W ko-group
    XG0 = FP8_KO // XG        # first bf16 x ko-group

    nc = bacc.Bacc("TRN2", target_bir_lowering=False)
    xT = nc.dram_tensor("xT", [D, T], bf16, kind="ExternalInput")
    ws = [
        nc.dram_tensor(f"w{j}", [HC, P, H], bf16, kind="ExternalInput")
        for j in range(K)
    ]
    if FP8_KO:
        x8T = nc.dram_tensor("x8T", [FP8_KO * P, T], f8, kind="ExternalInput")
        w8s = [
            nc.dram_tensor(f"w8_{j}", [HC, P, FP8_KO * P], f8, kind="ExternalInput")
            for j in range(K)
        ]
    y = nc.dram_tensor("y", [HC, P, T], bf16, kind="ExternalOutput")

    with tile.TileContext(nc) as tc:
        with (
            tc.tile_pool(name="warm", bufs=1) as warm,
            tc.tile_pool(name="xpool", bufs=1) as xpool,
            tc.tile_pool(name="wpool", bufs=4) as wpool,
            tc.tile_pool(name="ypool", bufs=4) as ypool,
            tc.tile_pool(name="psum", bufs=1, space="PSUM") as psum,
        ):
            # PE warm-up: dummies with no DMA deps run during the initial
            # fill window so HAM ramps the clock before real matmuls.
            wa = warm.tile([P, P], bf16)
            nc.vector.memset(wa[:], 0.0)
            nps = 4 if FP8_KO else 6
            pss = [
                psum.tile([P, 512], f32, name=f"ps{i}", tag=f"ps{i}")
                for i in range(nps)
            ]
            ps8s = [
                psum.tile([P, 512], f32, name=f"ps8_{i}", tag=f"ps8_{i}")
                for i in range(4 if FP8_KO else 0)
            ]
            for i in range(WARMUP):
                nc.tensor.matmul(
                    pss[i % nps][:, :P], wa[:], wa[:], start=True, stop=True
                )

            xr = xT.rearrange("(ko p) t -> p ko t", p=P)
            if FP8_KO:
                x8r = x8T.rearrange("(ko p) t -> p ko t", p=P)
            xt, x8t = {}, {}

            def x_issues_for_slot(j):
                """DMA-issue thunks for slot j's resident x tiles (bf16
                ko-groups >= XG0, plus fp8 pair tiles)."""
                issues = []
                for _, t0, w in wins_by_slot[j]:
                    for q in range(Q8):
                        def issue(t0=t0, w=w, q=q):
                            tl = xpool.tile([P, 2, w], f8, tag=f"x8_{t0}_{q}")
                            nc.sync.dma_start(
                                tl[:], x8r[:, 2 * q : 2 * q + 2, t0 : t0 + w]
                            )
                            x8t[(t0, q)] = tl
                        issues.append(issue)
                    for g in range(XG0, KO // XG):
                        def issue(t0=t0, w=w, g=g):
                            tl = xpool.tile([P, XG, w], bf16, tag=f"x_{t0}_{g}")
                            nc.sync.dma_start(
                                tl[:], xr[:, g * XG : (g + 1) * XG, t0 : t0 + w]
                            )
                            xt[(t0, g)] = tl
                        issues.append(issue)
                return issues

            rot = [0]
            y_pending = []

            def do_piece(h, t0, off, pw, wt, wt8, y_tag):
                """Matmuls + eviction + writeback for token range
                [t0+off, t0+off+pw) of the window starting at t0."""
                r = rot[0] % len(pss)
                rot[0] += 1
                ps = pss[r]
                if FP8_KO:
                    ps8 = ps8s[r]
                    for q in range(Q8):
                        nc.tensor.matmul(
                            ps8[:, :pw],
                            wt8[:, 2 * q : 2 * q + 2, :],
                            x8t[(t0, q)][:, :, off : off + pw],
                            start=(q == 0),
                            stop=(q == Q8 - 1),
                            perf_mode=mybir.MatmulPerfMode.DoubleRow,
                        )
                for ko in range(FP8_KO, KO):
                    nc.tensor.matmul(
                        ps[:, :pw],
                        wt[ko // KG][:, (ko % KG) * P : (ko % KG + 1) * P],
                        xt[(t0, ko // XG)][:, ko % XG, off : off + pw],
                        start=(ko == FP8_KO),
                        stop=(ko == KO - 1),
                    )
                if y_tag is None:
                    yt = ypool.tile([P, 512], bf16, tag="y")
                else:
                    yt = ypool.tile([P, pw], bf16, tag=y_tag)
                if FP8_KO:
                    nc.vector.tensor_scalar_mul(yt[:, :pw], ps8[:, :pw], descale)
                    nc.vector.tensor_tensor(
                        yt[:, :pw], yt[:, :pw], ps[:, :pw], mybir.AluOpType.add
                    )
                else:
                    nc.vector.tensor_copy(yt[:, :pw], ps[:, :pw])
                if y_tag is None:
                    nc.scalar.dma_start(y[h, :, t0 + off : t0 + off + pw], yt[:, :pw])
                else:
                    # Phase 0 is DMA-saturated (~270GB/s): park its y in
                    # SBUF and flush during phase 1, which has headroom.
                    y_pending.append((h, t0 + off, pw, yt))

            def do_windows(h, j, wt, wt8, buffer_y=False, split_last=False):
                wl = wins_by_slot[j]
                for wi, (_, t0, w) in enumerate(wl):
                    y_tag = f"y0_{h}_{wi}" if buffer_y else None
                    if split_last and wi == len(wl) - 1:
                        # Halve the final window so the first half's eviction
                        # and writeback overlap the second half's matmuls.
                        nc_half = w // 2
                        do_piece(h, t0, 0, nc_half, wt, wt8, y_tag)
                        do_piece(h, t0, nc_half, w - nc_half, wt, wt8, y_tag)
                    else:
                        do_piece(h, t0, 0, w, wt, wt8, y_tag)

            def flush_y(n):
                for _ in range(n):
                    if y_pending:
                        fh, ft0, fw, fyt = y_pending.pop(0)
                        nc.scalar.dma_start(y[fh, :, ft0 : ft0 + fw], fyt[:, :fw])

            def load_w(j, h):
                tls = {}
                for g in range(G0, NG):
                    tl = wpool.tile([P, KG * P], bf16, tag=f"w{j}_{g}")
                    nc.gpsimd.dma_start(
                        tl[:], ws[j][h, :, g * KG * P : (g + 1) * KG * P]
                    )
                    tls[g] = tl
                return tls

            def load_w8(j, h):
                tl = wpool.tile([P, FP8_KO, P], f8, tag=f"w8_{j}")
                nc.gpsimd.dma_start(
                    tl[:],
                    w8s[j][h].rearrange("p (ko hi) -> p ko hi", hi=P),
                )
                return tl

            # Slot 0's x tiles are needed immediately; later slots' issues are
            # spread across the back half of the preceding slot's h-scan so
            # the early queues carry only what the current phase consumes.
            for issue in x_issues_for_slot(0):
                issue()
            DEFER_H = 3  # first h-step of a phase that issues next-slot x
            for j in range(K):
                pending = x_issues_for_slot(j + 1) if j + 1 < K else []
                per_h = -(-len(pending) // (HC - DEFER_H)) if pending else 0
                n_flush = -(-len(y_pending) // HC) if j == 1 else 0
                for h in range(HC):
                    wt8 = load_w8(j, h) if FP8_KO else None
                    wt = load_w(j, h)
                    do_windows(
                        h, j, wt, wt8,
                        buffer_y=(j == 0 and K > 1),
                        split_last=(j == K - 1 and h == HC - 1),
                    )
                    flush_y(n_flush)
                    if h >= DEFER_H:
                        for _ in range(per_h):
                            if pending:
                                pending.pop(0)()
            flush_y(len(y_pending))
    nc.compile()
    return nc


def _install_profshim():
    """Register the NTFF profile hook trn_boot couldn't (image's antenv lacks
    axon_hooks) and stub the S3 artifact upload. Only needed when TRACE."""
    import sys
    import types

    import antenv

    if "antenv.axon_hooks" not in sys.modules:
        mod = types.ModuleType("antenv.axon_hooks")
        _hook = [None]
        mod.set_axon_ntff_profile_hook = lambda h: _hook.__setitem__(0, h)
        mod.get_axon_ntff_profile_hook = lambda: _hook[0]
        sys.modules["antenv.axon_hooks"] = mod
        antenv.axon_hooks = mod
        from trn_agent_boot.trn_boot import _ntff_profile_via_ctypes

        mod.set_axon_ntff_profile_hook(
            _ntff_profile_via_ctypes("/opt/axon/libaxon_pjrt.so")
        )
    import concourse.bass_utils as _bu

    _bu.upload_artifacts = lambda tmpdir: f"local:{tmpdir}"


def kernel(x, expert_W, expert_b, gate_W, gate_b):
    global last_exec_time_ns, last_trace_path
    import ml_dtypes

    from concourse.bass_utils import run_bass_kernel_spmd

    x = np.asarray(x, dtype=np.float32)
    expert_W = np.asarray(expert_W, dtype=np.float32)
    expert_b = np.asarray(expert_b, dtype=np.float32)
    gate_W = np.asarray(gate_W, dtype=np.float32)
    gate_b = np.asarray(gate_b, dtype=np.float32)

    topk_idx, topk_w = _routing(x, gate_W, gate_b)

    # Dispatch: token lists per expert (each token appears in exactly TOPK).
    tok = [np.nonzero((topk_idx == e).any(axis=1))[0] for e in range(E)]
    counts = np.array([len(t) for t in tok])
    T, sizes, assign = _plan_slots(counts)
    # Largest slot first: its h-scan phase is the longest window over which
    # the startup DMA transient (x + W streams) can hide.
    order = sorted(range(len(sizes)), key=lambda j: -sizes[j])
    sizes = tuple(sizes[j] for j in order)
    assign = tuple(tuple(row[j] for j in order) for row in assign)
    K = len(sizes)
    offs = np.concatenate([[0], np.cumsum(sizes)])

    # Slot -> expert per core: slot j's 8 core-slots are dealt to experts
    # per assign[:, j].
    slot_expert = np.zeros((NCORES, K), dtype=np.int64)
    for j in range(K):
        lst = [e for e in range(E) for _ in range(assign[e][j])]
        assert len(lst) == NCORES
        for c in range(NCORES):
            slot_expert[c, j] = lst[c]

    bf16 = ml_dtypes.bfloat16
    f8 = ml_dtypes.float8_e4m3
    xb = x.astype(bf16)
    # Power-of-2 fp8 scales sized so absmax lands just under e4m3 max (240).
    DC = FP8_KO * P
    if FP8_KO:
        sx = 2.0 ** math.floor(math.log2(224.0 / float(np.abs(x[:, :DC]).max())))
        sw = 2.0 ** math.floor(
            math.log2(224.0 / float(np.abs(expert_W[:, :, :DC]).max()))
        )
        descale = 1.0 / (sx * sw)
    else:
        sx = sw = descale = 1.0
    # W prepack per expert: [ho, p, ko*128+hi] = W_e[ho*128+hi, ko*128+p]
    wpack = [
        np.ascontiguousarray(
            expert_W[e]
            .reshape(HC, P, KO, P)
            .transpose(0, 3, 2, 1)
            .reshape(HC, P, H)
            .astype(bf16)
        )
        for e in range(E)
    ]
    w8pack = [
        np.ascontiguousarray(
            (expert_W[e][:, :DC] * sw)
            .reshape(HC, P, FP8_KO, P)
            .transpose(0, 3, 2, 1)
            .reshape(HC, P, DC)
            .astype(f8)
        )
        for e in range(E)
    ] if FP8_KO else None

    xTs = [np.zeros((D, T), dtype=bf16) for _ in range(NCORES)]
    x8s = [np.zeros((DC, T), dtype=f8) for _ in range(NCORES)] if FP8_KO else None
    core_of = np.zeros((E, B), dtype=np.int64)
    pos_of = np.zeros((E, B), dtype=np.int64)
    for e in range(E):
        pieces = [
            (c, j) for j in range(K) for c in range(NCORES) if slot_expert[c, j] == e
        ]
        cum = 0
        for c, j in pieces:
            n = min(int(sizes[j]), len(tok[e]) - cum)
            if n <= 0:
                continue
            t = tok[e][cum : cum + n]
            lo = int(offs[j])
            xTs[c][:, lo : lo + n] = xb[t].T
            if FP8_KO:
                x8s[c][:, lo : lo + n] = (x[t, :DC] * sx).astype(f8).T
            core_of[e, t] = c
            pos_of[e, t] = lo + np.arange(n)
            cum += n
        assert cum == len(tok[e]), f"expert {e}: assigned {cum} of {len(tok[e])}"

    in_maps = []
    for c in range(NCORES):
        m = {"xT": xTs[c]}
        for j in range(K):
            m[f"w{j}"] = wpack[slot_expert[c, j]]
            if FP8_KO:
                m[f"w8_{j}"] = w8pack[slot_expert[c, j]]
        if FP8_KO:
            m["x8T"] = x8s[c]
        in_maps.append(m)

    if TRACE:
        _install_profshim()
    nc = _build_bass(T, sizes, descale)
    res = run_bass_kernel_spmd(nc, in_maps, list(range(NCORES)), trace=TRACE)
    last_exec_time_ns = res.exec_time_ns
    if res.instructions_and_trace:
        last_trace_path = res.instructions_and_trace[1]

    # y [HC, P, T] -> [T, H] fp32 per core
    Ys = np.stack(
        [
            np.ascontiguousarray(
                np.asarray(res.results[c]["y"]).transpose(2, 0, 1)
            ).reshape(T, H).astype(np.float32)
            for c in range(NCORES)
        ]
    )

    barange = np.arange(B)
    out = np.zeros((B, H), dtype=np.float32)
    for k in range(TOPK):
        ek = topk_idx[:, k]
        out += topk_w[:, k, None] * (
            Ys[core_of[ek, barange], pos_of[ek, barange], :] + expert_b[ek]
        )
    return out
